# revision 1
# baseline (speedup 1.0000x reference)
"""CT-LSTM cell kernel for Trainium2, data-parallel over 8 NeuronCores.

Computes, for B=1048576 rows:
    z = [x, h_prev] @ W + b            (W = concat of 5 [80,16] mats -> [80,80])
    i, f, o, c~ = tanh(z[:, 0:64] split); decay = softplus(z[:, 64:80])
    c_next = f * (c_prev * exp(-decay*dt)) + i * c~
    h_next = o * tanh(c_next)

Layout strategy: x/h are passed feature-major (host-transposed) so the GEMM
stationary operand ([81, 128] slices, batch on the M axis) needs no on-device
transpose and produces batch-major z in PSUM.  c_prev/delta_t/outputs use a
partition-major [128, J, 16] host layout so every DMA is contiguous per
partition.  The softplus runs once per 16384-row mega-group so the ACT
table-set switches (exp/tanh set <-> softplus set) amortize.
"""

import sys

import numpy as np

sys.path.insert(0, "/opt/trn_rl_repo")

from concourse import bacc, bass, mybir, tile  # noqa: E402
from concourse.bass_utils import run_bass_kernel_spmd  # noqa: E402

F32 = mybir.dt.float32
AF = mybir.ActivationFunctionType
ALU = mybir.AluOpType

N_CORES = 8
BATCH = 1048576
R = BATCH // N_CORES  # rows per core = 131072
D_X = 64
D_H = 16
NG = 80  # 5 gates x 16


def build_program(rows, mega, chunk, n_cores=N_CORES):
    """Build + compile the Bass program (same NEFF for every core)."""
    assert rows % mega == 0 and mega % chunk == 0 and chunk % 512 == 0
    n_mega = rows // mega
    J = mega // 128  # subtiles (and free-dim groups) per mega-group
    n_chunk = mega // chunk
    sub_per_chunk = chunk // 128
    n_pt = sub_per_chunk // 4  # psum tiles (4 subtiles each) per chunk
    jcols = rows // 128

    nc = bacc.Bacc(
        "TRN2",
        target_bir_lowering=False,
        debug=False,
        num_devices=n_cores,
    )
    xT = nc.dram_tensor("xT", [D_X, rows], F32, kind="ExternalInput").ap()
    hT = nc.dram_tensor("hT", [D_H + 1, rows], F32, kind="ExternalInput").ap()
    cp = nc.dram_tensor("cp", [128, jcols, D_H], F32, kind="ExternalInput").ap()
    dt = nc.dram_tensor("dt", [128, jcols], F32, kind="ExternalInput").ap()
    wb = nc.dram_tensor("wb", [NG + 1, NG], F32, kind="ExternalInput").ap()
    ho = nc.dram_tensor("ho", [128, jcols, D_H], F32, kind="ExternalOutput").ap()
    co = nc.dram_tensor("co", [128, jcols, D_H], F32, kind="ExternalOutput").ap()

    with tile.TileContext(nc) as tc:
        with (
            tc.tile_pool(name="wbp", bufs=1) as wbp,
            tc.tile_pool(name="cmb", bufs=2) as cmb_pool,
            tc.tile_pool(name="psum", bufs=8, space="PSUM") as psum_pool,
            tc.tile_pool(name="gates", bufs=2) as gates_pool,
            tc.tile_pool(name="op", bufs=2) as o_pool,
            tc.tile_pool(name="zd", bufs=2) as zd_pool,
            tc.tile_pool(name="t2p", bufs=2) as t2_pool,
            tc.tile_pool(name="cpt", bufs=2) as cp_pool,
            tc.tile_pool(name="dtt", bufs=2) as dt_pool,
            tc.tile_pool(name="hout", bufs=1) as ho_pool,
        ):
            wb_t = wbp.tile([NG + 1, NG], F32)
            nc.sync.dma_start(wb_t[:], wb[:, :])

            # Software-pipelined emission: phase A (GEMM + drains + exp-set
            # ops) of group g is emitted BEFORE the decay chain of group
            # g-1, so the serial ACT<->DVE ping-pong of the chain hides
            # under the next group's dense PE/DMA/drain work.  Only the
            # Ln/Exp(-u) pair lives in the natural_log_exp table set; the
            # rest (tanh drains, exp(zd), tanh(c_next)) share exp_and_others
            # => still exactly 2 ACT table switches per mega-group.
            state = {}  # per-group tiles carried from phase A to the chain

            JH = J * D_H

            def r3(ap2d):
                # [128, n*16] flat view -> [128, n, 16]
                return ap2d.rearrange("p (a b) -> p a b", b=D_H)

            def phase_a(g):
                g0 = g * J
                # [*,16]-innermost tiles are allocated flat [128, n*16] —
                # a 3-D [.., 16] tile would pad the 64B inner dim to 128B
                # and double SBUF usage.
                cp_t = cp_pool.tile([128, JH], F32, tag="cp", name=f"cp{g}")
                nc.sync.dma_start(r3(cp_t[:]), cp[:, g0 : g0 + J, :])
                dt_t = dt_pool.tile([128, J], F32, tag="dt", name=f"dt{g}")
                nc.sync.dma_start(dt_t[:], dt[:, g0 : g0 + J])

                zdb = zd_pool.tile([128, JH], F32, tag="zd", name=f"zd{g}")
                t2 = t2_pool.tile([128, JH], F32, tag="t2", name=f"t2{g}")
                o_m = o_pool.tile([128, JH], F32, tag="om", name=f"o{g}")

                for c in range(n_chunk):
                    off = g * mega + c * chunk
                    cmbT = cmb_pool.tile([NG + 1, chunk], F32, name="cmbT")
                    nc.sync.dma_start(cmbT[0:D_X, :], xT[:, off : off + chunk])
                    nc.sync.dma_start(cmbT[D_X : NG + 1, :], hT[:, off : off + chunk])

                    # gates live only for one chunk; o is copied out to a
                    # mega-group buffer on the (otherwise idle) GPSIMD engine
                    gates = gates_pool.tile([128, sub_per_chunk, 64], F32,
                                            name="gates")
                    for t in range(n_pt):
                        ps = psum_pool.tile([128, 4, NG], F32, name="ps")
                        for jj in range(4):
                            col = (t * 4 + jj) * 128
                            nc.tensor.matmul(
                                ps[:, jj, :],
                                lhsT=cmbT[:, col : col + 128],
                                rhs=wb_t[:],
                                start=True,
                                stop=True,
                            )
                        jb = (c * sub_per_chunk + t * 4) * D_H
                        nc.scalar.activation(
                            gates[:, t * 4 : t * 4 + 4, :], ps[:, :, 0:64], AF.Tanh
                        )
                        nc.vector.tensor_copy(
                            r3(zdb[:, jb : jb + 4 * D_H]), ps[:, :, 64:NG]
                        )
                    # chunk-level exp-set / DVE work pulled off the chain
                    # (in-place: zdb <- exp(zd), cp_t <- f * c_prev)
                    cf = slice(c * sub_per_chunk * D_H, (c + 1) * sub_per_chunk * D_H)
                    nc.scalar.activation(zdb[:, cf], zdb[:, cf], AF.Exp)
                    nc.vector.tensor_tensor(
                        r3(t2[:, cf]), gates[:, :, 0:16], gates[:, :, 48:64],
                        ALU.mult,
                    )
                    nc.vector.tensor_tensor(
                        r3(cp_t[:, cf]), gates[:, :, 16:32], r3(cp_t[:, cf]),
                        ALU.mult,
                    )
                    nc.gpsimd.tensor_copy(r3(o_m[:, cf]), gates[:, :, 32:48])
                state[g] = (cp_t, dt_t, o_m, zdb, t2)

            def chain(g):
                g0 = g * J
                cp_t, dt_t, o_m, zdb, t2 = state.pop(g)
                # softplus(zd) = ln(1 + exp(zd)) — Ln and Exp(-u) share the
                # natural_log_exp_and_others table set (one contiguous
                # ln-set window per mega-group).  All steps run in place on
                # zdb / cp_t to keep SBUF within budget:
                #   zdb: exp(zd) -> s=ln(1+.) -> u=s*dt -> E=exp(-u)
                #   cp_t: f*c_prev -> t1=(f*c_prev)*E
                dt_b = dt_t[:].unsqueeze(2).broadcast_to((128, J, D_H))
                nc.scalar.activation(zdb[:], zdb[:], AF.Ln, bias=1.0)
                nc.vector.tensor_tensor(r3(zdb[:]), r3(zdb[:]), dt_b, ALU.mult)
                nc.scalar.activation(zdb[:], zdb[:], AF.Exp, scale=-1.0)
                nc.vector.tensor_tensor(cp_t[:], cp_t[:], zdb[:], ALU.mult)
                # c_next lands in cp_t's buffer (dead after this add);
                # t2 is dead after the add too and holds tanh(c_next)
                nc.vector.tensor_tensor(cp_t[:], cp_t[:], t2[:], ALU.add)
                nc.scalar.activation(t2[:], cp_t[:], AF.Tanh)
                ho_t = ho_pool.tile([128, JH], F32, tag="ho", name=f"ho{g}")
                nc.vector.tensor_tensor(ho_t[:], o_m[:], t2[:], ALU.mult)

                nc.sync.dma_start(ho[:, g0 : g0 + J, :], r3(ho_t[:]))
                nc.sync.dma_start(co[:, g0 : g0 + J, :], r3(cp_t[:]))

            for g in range(n_mega + 1):
                if g < n_mega:
                    phase_a(g)
                if g >= 1:
                    chain(g - 1)

    nc.compile()
    return nc


def marshal_core_inputs(x, h_prev, c_prev, delta_t, wb_np, lo, hi):
    """Build one core's input map from a batch slice [lo, hi)."""
    rows = hi - lo
    nm = rows // 128  # j-columns
    xs = np.ascontiguousarray(x[lo:hi].T)  # [64, rows]
    hs = np.empty((D_H + 1, rows), np.float32)
    hs[:D_H] = h_prev[lo:hi].T
    hs[D_H] = 1.0  # bias row
    # device row (p, jcol) <-> original row jcol*128 + p
    cps = np.ascontiguousarray(
        c_prev[lo:hi].reshape(nm, 128, D_H).transpose(1, 0, 2)
    )  # [128, nm, 16]
    dts = np.ascontiguousarray(delta_t[lo:hi].reshape(nm, 128).T)  # [128, nm]
    return {"xT": xs, "hT": hs, "cp": cps, "dt": dts, "wb": wb_np}


def unmarshal_output(dev_out, rows):
    """[128, nm, 16] partition-major -> [rows, 16] batch-major."""
    nm = rows // 128
    return np.ascontiguousarray(dev_out.transpose(1, 0, 2).reshape(rows, D_H))


_PROGRAM_CACHE = {}


def _get_program(rows, mega, chunk):
    key = (rows, mega, chunk)
    if key not in _PROGRAM_CACHE:
        _PROGRAM_CACHE[key] = build_program(rows, mega, chunk)
    return _PROGRAM_CACHE[key]


def run(x, h_prev, c_prev, delta_t, wb_np, rows_per_core, mega, chunk, trace=False):
    nc = _get_program(rows_per_core, mega, chunk)
    n_cores = N_CORES
    in_maps = [
        marshal_core_inputs(
            x, h_prev, c_prev, delta_t, wb_np,
            i * rows_per_core, (i + 1) * rows_per_core,
        )
        for i in range(n_cores)
    ]
    res = run_bass_kernel_spmd(nc, in_maps, list(range(n_cores)), trace=trace)
    h_parts = [unmarshal_output(res.results[i]["ho"], rows_per_core) for i in range(n_cores)]
    c_parts = [unmarshal_output(res.results[i]["co"], rows_per_core) for i in range(n_cores)]
    h_next = np.concatenate(h_parts, axis=0)
    c_next = np.concatenate(c_parts, axis=0)
    return (h_next, c_next), res


def kernel(x, h_prev, c_prev, delta_t, W_i, b_i, W_f, b_f, W_o, b_o, W_c, b_c, W_d, b_d):
    x = np.asarray(x, np.float32)
    h_prev = np.asarray(h_prev, np.float32)
    c_prev = np.asarray(c_prev, np.float32)
    delta_t = np.asarray(delta_t, np.float32)
    W = np.concatenate(
        [np.asarray(w, np.float32) for w in (W_i, W_f, W_o, W_c, W_d)], axis=1
    )  # [80, 80]
    b = np.concatenate(
        [np.asarray(v, np.float32) for v in (b_i, b_f, b_o, b_c, b_d)]
    )  # [80]
    wb_np = np.ascontiguousarray(np.vstack([W, b[None, :]]))  # [81, 80]

    (h_next, c_next), _ = run(
        x, h_prev, c_prev, delta_t, wb_np,
        rows_per_core=R, mega=16384, chunk=4096,
    )
    return (h_next, c_next)



# revision 23
# speedup vs baseline: 1.7323x; 1.7323x over previous
"""CT-LSTM cell kernel for Trainium2, data-parallel over 8 NeuronCores.

Computes, for B=1048576 rows:
    z = [x, h_prev] @ W + b            (W = concat of 5 [80,16] mats -> [80,80])
    i, f, o, c~ = tanh(z[:, 0:64] split); decay = softplus(z[:, 64:80])
    c_next = f * (c_prev * exp(-decay*dt)) + i * c~
    h_next = o * tanh(c_next)

v4 strategy (vs fp32 baseline):
  * All I/O and SBUF elementwise tensors are fp16 (halves DMA bytes, 1
    cycle/row matmuls, 2x DVE mode); PSUM accumulates fp32; softplus via
    exp + ln(1+x) (AF.Softplus has no table on this stack).
  * PSUM per 2048-row chunk: gate matmuls fill a [128, 2, 512] tile (8
    subtiles x 64 cols fill each 2KB bank exactly -> ONE contiguous
    [128,1024] tanh drain into a mega-resident fp16 gates buffer); the 16
    decay cols go to a separate bank, DVE-copied to fp16.
  * The decay/cell chain runs at mega scope in 2 sub-slices to cut its
    serial latency; both outputs are packed into one [128, J, 32] tile and
    written by a single DMA per sub-slice.
  * DMA holds (the issuing sequencer is busy for the WHOLE transfer incl.
    waits in the cost model) are split: inputs on SP, outputs on Pool
    (SWDGE); cp/dt are emitted after the x/h slabs so SP's holds are
    short.  ACT never issues DMAs - it is the bottleneck engine.
"""

import sys

import numpy as np

sys.path.insert(0, "/opt/trn_rl_repo")

from concourse import bacc, bass, mybir, tile  # noqa: E402
from concourse.bass_utils import run_bass_kernel_spmd  # noqa: E402

F32 = mybir.dt.float32
F16 = mybir.dt.float16
AF = mybir.ActivationFunctionType
ALU = mybir.AluOpType

N_CORES = 8
BATCH = 1048576
R = BATCH // N_CORES  # rows per core = 131072
D_X = 64
D_H = 16
KD = D_X + D_H + 1  # 81 contraction rows (incl. bias row)
import os as _os

N_SLICE = int(_os.environ.get("K_NSLICE", "4"))  # chain sub-slices per mega
DMACHUNK = int(_os.environ.get("K_DMACHUNK", "4096"))


def build_program(rows, mega, chunk, n_cores=N_CORES):
    """Build + compile the Bass program (same NEFF for every core)."""
    assert rows % mega == 0 and mega % chunk == 0 and chunk == 2048
    n_mega = rows // mega
    J = mega // 128  # subtiles per mega-group
    JH = J * D_H
    n_chunk = mega // chunk  # chunks per mega-group
    spc = chunk // 128  # subtiles per chunk = 16
    dmachunk = min(DMACHUNK, mega)
    n_dma = mega // dmachunk
    cpd = dmachunk // chunk  # chunks per dma slab
    jcols = rows // 128
    assert J % N_SLICE == 0
    JS = J // N_SLICE  # subtiles per chain slice

    nc = bacc.Bacc(
        "TRN2",
        target_bir_lowering=False,
        debug=False,
        num_devices=n_cores,
    )
    xT = nc.dram_tensor("xT", [D_X, rows], F16, kind="ExternalInput").ap()
    hT = nc.dram_tensor("hT", [D_H + 1, rows], F16, kind="ExternalInput").ap()
    cp = nc.dram_tensor("cp", [128, jcols, D_H], F16, kind="ExternalInput").ap()
    dt = nc.dram_tensor("dt", [128, jcols], F16, kind="ExternalInput").ap()
    w64 = nc.dram_tensor("w64", [KD, 64], F16, kind="ExternalInput").ap()
    w16 = nc.dram_tensor("w16", [KD, D_H], F16, kind="ExternalInput").ap()
    # packed output: [..., 0:16] = h_next, [..., 16:32] = c_next
    hc = nc.dram_tensor("hc", [128, jcols, 2 * D_H], F16, kind="ExternalOutput").ap()

    with tile.TileContext(nc) as tc:
        with (
            tc.tile_pool(name="wbp", bufs=1) as wbp,
            tc.tile_pool(name="cmb", bufs=2) as cmb_pool,
            tc.tile_pool(name="psG", bufs=2, space="PSUM") as psG_pool,
            tc.tile_pool(name="psD", bufs=2, space="PSUM") as psD_pool,
            tc.tile_pool(name="gates", bufs=2) as gates_pool,
            tc.tile_pool(name="dtb", bufs=2) as dtb_pool,
            tc.tile_pool(name="zd", bufs=2) as zd_pool,
            tc.tile_pool(name="cpt", bufs=2) as cp_pool,
            tc.tile_pool(name="dtt", bufs=2) as dt_pool,
            tc.tile_pool(name="hcout", bufs=2) as hc_pool,
        ):
            w64_t = wbp.tile([KD, 64], F16)
            nc.sync.dma_start(w64_t[:], w64[:, :])
            w16_t = wbp.tile([KD, D_H], F16)
            nc.sync.dma_start(w16_t[:], w16[:, :])

            # Software-pipelined emission: phase A (DMA + GEMM + drains) of
            # group g is emitted BEFORE the decay chain of group g-1 so the
            # serial chain hides under dense work.
            state = {}

            def r3(ap2d, inner=D_H):
                return ap2d.rearrange("p (a b) -> p a b", b=inner)

            def phase_a(g):
                g0 = g * J
                zdb = zd_pool.tile([128, JH], F16, tag="zd", name=f"zd{g}")
                gates = gates_pool.tile([128, J * 64], F16, tag="gt",
                                        name=f"gt{g}")

                for d in range(n_dma):
                    off = g * mega + d * dmachunk
                    cmbT = cmb_pool.tile([KD, dmachunk], F16, name="cmbT")
                    nc.sync.dma_start(cmbT[0:D_X, :], xT[:, off : off + dmachunk])
                    nc.sync.dma_start(
                        cmbT[D_X:KD, :], hT[:, off : off + dmachunk]
                    )
                    for c2 in range(cpd):
                        c = d * cpd + c2  # chunk index within mega-group
                        psG = psG_pool.tile([128, 2, 512], F32, name="psG")
                        psD = psD_pool.tile([128, 512], F32, name="psD")
                        for j in range(spc):
                            col = c2 * chunk + j * 128
                            lt = cmbT[:, col : col + 128]
                            nc.tensor.matmul(
                                psG[:, j // 8, 64 * (j % 8) : 64 * (j % 8) + 64],
                                lhsT=lt,
                                rhs=w64_t[:],
                                start=True,
                                stop=True,
                            )
                            nc.tensor.matmul(
                                psD[:, D_H * j : D_H * j + D_H],
                                lhsT=lt,
                                rhs=w16_t[:],
                                start=True,
                                stop=True,
                            )
                        nc.scalar.activation(
                            gates[:, c * spc * 64 : (c + 1) * spc * 64],
                            psG[:].rearrange("p a b -> p (a b)"),
                            AF.Tanh,
                        )
                        nc.vector.tensor_copy(
                            zdb[:, c * spc * D_H : (c + 1) * spc * D_H],
                            psD[:, 0 : spc * D_H],
                        )
                # cp/dt after the x/h slabs: they're needed only by the
                # chain, and late emission keeps SP's DMA holds short.
                cp_t = cp_pool.tile([128, JH], F16, tag="cp", name=f"cp{g}")
                nc.sync.dma_start(r3(cp_t[:]), cp[:, g0 : g0 + J, :])
                dt_t = dt_pool.tile([128, J], F16, tag="dt", name=f"dt{g}")
                nc.sync.dma_start(dt_t[:], dt[:, g0 : g0 + J])
                # Pre-broadcast dt to [128, J, 16] on Pool (off the critical
                # path) so the chain's u-mult is a contiguous 2x DVE op
                # instead of a slow strided-broadcast mult.
                dtb_t = dtb_pool.tile([128, JH], F16, tag="dtb", name=f"dtb{g}")
                nc.gpsimd.tensor_copy(
                    r3(dtb_t[:]),
                    dt_t[:].unsqueeze(2).broadcast_to((128, J, D_H)),
                )
                state[g] = (cp_t, dtb_t, gates, zdb)

            flush = {}

            def do_flush(g):
                # Output DMAs for group g are emitted one iteration after
                # chain(g) computed them, so the Pool sequencer never waits
                # on the chain: the data is long since ready.
                g0 = g * J
                hc_t = flush.pop(g)
                hc3 = r3(hc_t[:], inner=2 * D_H)
                for s in range(N_SLICE):
                    js = slice(s * JS, (s + 1) * JS)
                    nc.gpsimd.dma_start(
                        hc[:, g0 + s * JS : g0 + (s + 1) * JS, :],
                        hc3[:, js, :],
                    )

            def chain(g):
                g0 = g * J
                cp_t, dtb_t, gates, zdb = state.pop(g)
                if g >= 1:
                    do_flush(g - 1)
                # softplus(zd) = ln(1 + exp(zd)), full-mega ops (one
                # natural_log table window per mega-group)
                nc.scalar.activation(zdb[:], zdb[:], AF.Exp)
                nc.scalar.activation(zdb[:], zdb[:], AF.Ln, bias=1.0)
                hc_t = hc_pool.tile([128, J * 2 * D_H], F16, tag="hc",
                                    name=f"hc{g}")
                g4 = r3(gates[:], inner=64)
                hc3 = r3(hc_t[:], inner=2 * D_H)
                for s in range(N_SLICE):
                    js = slice(s * JS, (s + 1) * JS)
                    fs = slice(s * JS * D_H, (s + 1) * JS * D_H)
                    zs3 = r3(zdb[:, fs])
                    cps3 = r3(cp_t[:, fs])
                    hs3 = hc3[:, js, 0:D_H]
                    cs3 = hc3[:, js, D_H : 2 * D_H]
                    # u = sp * dt (DVE 2x), E = exp(-u) (ACT, shares the
                    # tanh table set), then cell update on DVE:
                    nc.vector.tensor_tensor(
                        zdb[:, fs], zdb[:, fs], dtb_t[:, fs], ALU.mult
                    )
                    nc.scalar.activation(zdb[:, fs], zdb[:, fs], AF.Exp,
                                         scale=-1.0)
                    # c_tilde*i into the c_next output slot
                    nc.vector.tensor_tensor(
                        cs3, g4[:, js, 0:16], g4[:, js, 48:64], ALU.mult
                    )
                    # f*c_prev, then *E (both in place on cp_t)
                    nc.vector.tensor_tensor(
                        cps3, g4[:, js, 16:32], cps3, ALU.mult
                    )
                    nc.vector.tensor_tensor(cps3, cps3, zs3, ALU.mult)
                    # c_next = f*c_decay + i*c~
                    nc.vector.tensor_tensor(cs3, cs3, cps3, ALU.add)
                    # tanh(c_next) -> reuse zdb slice (E is dead)
                    nc.scalar.activation(zdb[:, fs], hc_t[:].rearrange(
                        "p (a b) -> p a b", b=2 * D_H)[:, js, D_H : 2 * D_H],
                        AF.Tanh)
                    nc.vector.tensor_tensor(
                        hs3, g4[:, js, 32:48], zs3, ALU.mult
                    )
                flush[g] = hc_t

            for g in range(n_mega + 1):
                if g < n_mega:
                    phase_a(g)
                if g >= 1:
                    chain(g - 1)
            do_flush(n_mega - 1)

    nc.compile()
    return nc


def marshal_core_inputs(x, h_prev, c_prev, delta_t, w64_np, w16_np, lo, hi):
    """Build one core's input map from a batch slice [lo, hi)."""
    rows = hi - lo
    nm = rows // 128
    xs = np.ascontiguousarray(x[lo:hi].T.astype(np.float16))
    hs = np.empty((D_H + 1, rows), np.float16)
    hs[:D_H] = h_prev[lo:hi].T
    hs[D_H] = 1.0  # bias row
    # device row (p, jcol) <-> original row jcol*128 + p
    cps = np.ascontiguousarray(
        c_prev[lo:hi].astype(np.float16).reshape(nm, 128, D_H).transpose(1, 0, 2)
    )
    dts = np.ascontiguousarray(delta_t[lo:hi].astype(np.float16).reshape(nm, 128).T)
    return {"xT": xs, "hT": hs, "cp": cps, "dt": dts, "w64": w64_np, "w16": w16_np}


def unmarshal_output(dev_out, rows):
    """[128, nm, 32] packed fp16 -> ([rows,16], [rows,16]) fp32 batch-major."""
    out = np.asarray(dev_out, np.float32).transpose(1, 0, 2).reshape(rows, 2 * D_H)
    return np.ascontiguousarray(out[:, :D_H]), np.ascontiguousarray(out[:, D_H:])


_PROGRAM_CACHE = {}


def _get_program(rows, mega, chunk):
    key = (rows, mega, chunk)
    if key not in _PROGRAM_CACHE:
        _PROGRAM_CACHE[key] = build_program(rows, mega, chunk)
    return _PROGRAM_CACHE[key]


def run(x, h_prev, c_prev, delta_t, w64_np, w16_np, rows_per_core, mega, chunk,
        trace=False):
    nc = _get_program(rows_per_core, mega, chunk)
    n_cores = N_CORES
    in_maps = [
        marshal_core_inputs(
            x, h_prev, c_prev, delta_t, w64_np, w16_np,
            i * rows_per_core, (i + 1) * rows_per_core,
        )
        for i in range(n_cores)
    ]
    res = run_bass_kernel_spmd(nc, in_maps, list(range(n_cores)), trace=trace)
    parts = [unmarshal_output(res.results[i]["hc"], rows_per_core) for i in range(n_cores)]
    h_next = np.concatenate([p[0] for p in parts], axis=0)
    c_next = np.concatenate([p[1] for p in parts], axis=0)
    return (h_next, c_next), res


def make_weights(W_i, b_i, W_f, b_f, W_o, b_o, W_c, b_c, W_d, b_d):
    """[81,64] fp16 gates block + [81,16] fp16 decay block (bias rows last)."""
    W4 = np.concatenate(
        [np.asarray(w, np.float32) for w in (W_i, W_f, W_o, W_c)], axis=1
    )  # [80, 64]
    b4 = np.concatenate([np.asarray(v, np.float32) for v in (b_i, b_f, b_o, b_c)])
    w64_np = np.ascontiguousarray(
        np.vstack([W4, b4[None, :]]).astype(np.float16)
    )  # [81, 64]
    w16_np = np.ascontiguousarray(
        np.vstack([np.asarray(W_d, np.float32),
                   np.asarray(b_d, np.float32)[None, :]]).astype(np.float16)
    )  # [81, 16]
    return w64_np, w16_np


def kernel(x, h_prev, c_prev, delta_t, W_i, b_i, W_f, b_f, W_o, b_o, W_c, b_c, W_d, b_d):
    x = np.asarray(x, np.float32)
    h_prev = np.asarray(h_prev, np.float32)
    c_prev = np.asarray(c_prev, np.float32)
    delta_t = np.asarray(delta_t, np.float32)
    w64_np, w16_np = make_weights(
        W_i, b_i, W_f, b_f, W_o, b_o, W_c, b_c, W_d, b_d
    )
    (h_next, c_next), _ = run(
        x, h_prev, c_prev, delta_t, w64_np, w16_np,
        rows_per_core=R, mega=32768, chunk=2048,
    )
    return (h_next, c_next)


# revision 32
# speedup vs baseline: 1.9209x; 1.1089x over previous
"""CT-LSTM cell kernel for Trainium2, data-parallel over 8 NeuronCores.

Computes, for B=1048576 rows:
    z = [x, h_prev] @ W + b            (W = concat of 5 [80,16] mats -> [80,80])
    i, f, o, c~ = tanh(z[:, 0:64] split); decay = softplus(z[:, 64:80])
    c_next = f * (c_prev * exp(-decay*dt)) + i * c~
    h_next = o * tanh(c_next)

v4 strategy (vs fp32 baseline):
  * All I/O and SBUF elementwise tensors are fp16 (halves DMA bytes, 1
    cycle/row matmuls, 2x DVE mode); PSUM accumulates fp32; softplus via
    exp + ln(1+x) (AF.Softplus has no table on this stack).
  * PSUM per 2048-row chunk: gate matmuls fill a [128, 2, 512] tile (8
    subtiles x 64 cols fill each 2KB bank exactly -> ONE contiguous
    [128,1024] tanh drain into a mega-resident fp16 gates buffer); the 16
    decay cols go to a separate bank, DVE-copied to fp16.
  * The decay/cell chain runs at mega scope in 2 sub-slices to cut its
    serial latency; both outputs are packed into one [128, J, 32] tile and
    written by a single DMA per sub-slice.
  * DMA holds (the issuing sequencer is busy for the WHOLE transfer incl.
    waits in the cost model) are split: inputs on SP, outputs on Pool
    (SWDGE); cp/dt are emitted after the x/h slabs so SP's holds are
    short.  ACT never issues DMAs - it is the bottleneck engine.
"""

import sys

import numpy as np

sys.path.insert(0, "/opt/trn_rl_repo")

from concourse import bacc, bass, mybir, tile  # noqa: E402
from concourse.bass_utils import run_bass_kernel_spmd  # noqa: E402

F32 = mybir.dt.float32
F16 = mybir.dt.float16
AF = mybir.ActivationFunctionType
ALU = mybir.AluOpType

N_CORES = 8
BATCH = 1048576
R = BATCH // N_CORES  # rows per core = 131072
D_X = 64
D_H = 16
KD = D_X + D_H + 1  # 81 contraction rows (incl. bias row)
import os as _os

N_SLICE = int(_os.environ.get("K_NSLICE", "4"))  # chain sub-slices per mega
DMACHUNK = int(_os.environ.get("K_DMACHUNK", "4096"))
GSUB = 24  # subtiles per gate psum group (3 banks x 8 subtiles)
DSUB = 32  # subtiles per decay psum bank (32 x 16 cols = 2KB)


def build_program(rows, mega, chunk, n_cores=N_CORES):
    """Build + compile the Bass program (same NEFF for every core)."""
    assert rows % mega == 0 and mega % chunk == 0 and chunk == 2048
    n_mega = rows // mega
    J = mega // 128  # subtiles per mega-group
    JH = J * D_H
    n_chunk = mega // chunk  # chunks per mega-group
    spc = chunk // 128  # subtiles per chunk = 16
    dmachunk = min(DMACHUNK, mega)
    n_dma = mega // dmachunk
    cpd = dmachunk // chunk  # chunks per dma slab
    jcols = rows // 128
    assert J % N_SLICE == 0
    JS = J // N_SLICE  # subtiles per chain slice

    nc = bacc.Bacc(
        "TRN2",
        target_bir_lowering=False,
        debug=False,
        num_devices=n_cores,
    )
    xT = nc.dram_tensor("xT", [D_X, rows], F16, kind="ExternalInput").ap()
    hT = nc.dram_tensor("hT", [D_H + 1, rows], F16, kind="ExternalInput").ap()
    cp = nc.dram_tensor("cp", [128, jcols, D_H], F16, kind="ExternalInput").ap()
    dt = nc.dram_tensor("dt", [128, jcols], F16, kind="ExternalInput").ap()
    w64 = nc.dram_tensor("w64", [KD, 64], F16, kind="ExternalInput").ap()
    w16 = nc.dram_tensor("w16", [KD, D_H], F16, kind="ExternalInput").ap()
    # packed output: [..., 0:16] = h_next, [..., 16:32] = c_next
    hc = nc.dram_tensor("hc", [128, jcols, 2 * D_H], F16, kind="ExternalOutput").ap()

    with tile.TileContext(nc) as tc:
        with (
            tc.tile_pool(name="wbp", bufs=1) as wbp,
            tc.tile_pool(name="cmb", bufs=int(_os.environ.get("K_CMBBUFS", "4"))) as cmb_pool,
            tc.tile_pool(name="psG", bufs=2, space="PSUM") as psG_pool,
            tc.tile_pool(name="psD", bufs=2, space="PSUM") as psD_pool,
            tc.tile_pool(name="gates", bufs=2) as gates_pool,
            tc.tile_pool(name="dtb", bufs=2) as dtb_pool,
            tc.tile_pool(name="zd", bufs=2) as zd_pool,
            tc.tile_pool(name="cpt", bufs=2) as cp_pool,
            tc.tile_pool(name="dtt", bufs=2) as dt_pool,
            tc.tile_pool(name="hcout", bufs=2) as hc_pool,
        ):
            # weights ride the Pool queue so SP can start the first x/h
            # slabs immediately
            w64_t = wbp.tile([KD, 64], F16)
            nc.gpsimd.dma_start(w64_t[:], w64[:, :])
            w16_t = wbp.tile([KD, D_H], F16)
            nc.gpsimd.dma_start(w16_t[:], w16[:, :])

            # Software-pipelined emission: phase A (DMA + GEMM + drains) of
            # group g is emitted BEFORE the decay chain of group g-1 so the
            # serial chain hides under dense work.
            state = {}

            def r3(ap2d, inner=D_H):
                return ap2d.rearrange("p (a b) -> p a b", b=inner)

            def phase_a(g):
                g0 = g * J
                zdb = zd_pool.tile([128, JH], F16, tag="zd", name=f"zd{g}")
                gates = gates_pool.tile([128, J * 64], F16, tag="gt",
                                        name=f"gt{g}")

                # PSUM: gate groups of 24 subtiles (3 banks, 8 x 64 cols
                # filling each bank exactly) + decay banks of 32 subtiles
                # (32 x 16 cols = one full bank); 3*2 + 1*2 = 8 banks.
                # Groups are decoupled from the DMA slab structure: each
                # matmul reads whichever cmbT slab holds its columns.
                spst = dmachunk // 128  # subtiles per dma slab
                cmbT = None
                psG = None
                psD = None
                g_start = 0
                g_len = 0
                for s in range(J):
                    if s % spst == 0:
                        off = g * mega + (s // spst) * dmachunk
                        cmbT = cmb_pool.tile([KD, dmachunk], F16, name="cmbT")
                        nc.sync.dma_start(
                            cmbT[0:D_X, :], xT[:, off : off + dmachunk]
                        )
                        nc.sync.dma_start(
                            cmbT[D_X:KD, :], hT[:, off : off + dmachunk]
                        )
                    if psG is None:
                        g_start = s
                        g_len = min(GSUB, J - s)
                        psG = psG_pool.tile([128, 3, 512], F32, name="psG")
                    if s % DSUB == 0:
                        psD = psD_pool.tile([128, 512], F32, name="psD")
                    ls = s - g_start
                    col = (s % spst) * 128
                    lt = cmbT[:, col : col + 128]
                    nc.tensor.matmul(
                        psG[:, ls // 8, 64 * (ls % 8) : 64 * (ls % 8) + 64],
                        lhsT=lt,
                        rhs=w64_t[:],
                        start=True,
                        stop=True,
                    )
                    nc.tensor.matmul(
                        psD[:, D_H * (s % DSUB) : D_H * (s % DSUB) + D_H],
                        lhsT=lt,
                        rhs=w16_t[:],
                        start=True,
                        stop=True,
                    )
                    if ls == g_len - 1:
                        nbank = (g_len * 64) // 512
                        nc.scalar.activation(
                            gates[:, g_start * 64 : (g_start + g_len) * 64],
                            psG[:, 0:nbank, :].rearrange("p a b -> p (a b)"),
                            AF.Tanh,
                        )
                        psG = None
                    if s % DSUB == DSUB - 1:
                        nc.vector.tensor_copy(
                            zdb[:, (s - DSUB + 1) * D_H : (s + 1) * D_H],
                            psD[:],
                        )
                # cp/dt after the x/h slabs: they're needed only by the
                # chain, and late emission keeps SP's DMA holds short.
                cp_t = cp_pool.tile([128, JH], F16, tag="cp", name=f"cp{g}")
                nc.sync.dma_start(r3(cp_t[:]), cp[:, g0 : g0 + J, :])
                dt_t = dt_pool.tile([128, J], F16, tag="dt", name=f"dt{g}")
                nc.sync.dma_start(dt_t[:], dt[:, g0 : g0 + J])
                # Pre-broadcast dt to [128, J, 16] on Pool (off the critical
                # path) so the chain's u-mult is a contiguous 2x DVE op
                # instead of a slow strided-broadcast mult.
                dtb_t = dtb_pool.tile([128, JH], F16, tag="dtb", name=f"dtb{g}")
                nc.gpsimd.tensor_copy(
                    r3(dtb_t[:]),
                    dt_t[:].unsqueeze(2).broadcast_to((128, J, D_H)),
                )
                state[g] = (cp_t, dtb_t, gates, zdb)

            flush = {}

            def do_flush(g, final=False):
                # Output DMAs for group g are emitted one iteration after
                # chain(g) computed them, so the Pool sequencer never waits
                # on the chain: the data is long since ready.  The final
                # flush has no work to hide under, so split it across the
                # Pool and SP queues to halve the tail.
                g0 = g * J
                hc_t = flush.pop(g)
                hc3 = r3(hc_t[:], inner=2 * D_H)
                for s in range(N_SLICE):
                    js = slice(s * JS, (s + 1) * JS)
                    eng = nc.sync if (final and s % 2) else nc.gpsimd
                    eng.dma_start(
                        hc[:, g0 + s * JS : g0 + (s + 1) * JS, :],
                        hc3[:, js, :],
                    )

            def chain(g):
                g0 = g * J
                cp_t, dtb_t, gates, zdb = state.pop(g)
                if g >= 1:
                    do_flush(g - 1)
                # softplus(zd) = ln(1 + exp(zd)), full-mega ops (one
                # natural_log table window per mega-group)
                nc.scalar.activation(zdb[:], zdb[:], AF.Exp)
                nc.scalar.activation(zdb[:], zdb[:], AF.Ln, bias=1.0)
                hc_t = hc_pool.tile([128, J * 2 * D_H], F16, tag="hc",
                                    name=f"hc{g}")
                g4 = r3(gates[:], inner=64)
                hc3 = r3(hc_t[:], inner=2 * D_H)
                for s in range(N_SLICE):
                    js = slice(s * JS, (s + 1) * JS)
                    fs = slice(s * JS * D_H, (s + 1) * JS * D_H)
                    zs3 = r3(zdb[:, fs])
                    cps3 = r3(cp_t[:, fs])
                    hs3 = hc3[:, js, 0:D_H]
                    cs3 = hc3[:, js, D_H : 2 * D_H]
                    # u = sp * dt (DVE 2x), E = exp(-u) (ACT, shares the
                    # tanh table set), then cell update on DVE:
                    nc.vector.tensor_tensor(
                        zdb[:, fs], zdb[:, fs], dtb_t[:, fs], ALU.mult
                    )
                    nc.scalar.activation(zdb[:, fs], zdb[:, fs], AF.Exp,
                                         scale=-1.0)
                    # c_tilde*i into the c_next output slot
                    nc.vector.tensor_tensor(
                        cs3, g4[:, js, 0:16], g4[:, js, 48:64], ALU.mult
                    )
                    # f*c_prev, then *E (both in place on cp_t)
                    nc.vector.tensor_tensor(
                        cps3, g4[:, js, 16:32], cps3, ALU.mult
                    )
                    nc.vector.tensor_tensor(cps3, cps3, zs3, ALU.mult)
                    # c_next = f*c_decay + i*c~
                    nc.vector.tensor_tensor(cs3, cs3, cps3, ALU.add)
                    # tanh(c_next) -> reuse zdb slice (E is dead)
                    nc.scalar.activation(zdb[:, fs], hc_t[:].rearrange(
                        "p (a b) -> p a b", b=2 * D_H)[:, js, D_H : 2 * D_H],
                        AF.Tanh)
                    nc.vector.tensor_tensor(
                        hs3, g4[:, js, 32:48], zs3, ALU.mult
                    )
                flush[g] = hc_t

            for g in range(n_mega + 1):
                if g < n_mega:
                    phase_a(g)
                if g >= 1:
                    chain(g - 1)
            do_flush(n_mega - 1, final=True)

    nc.compile()
    return nc


def marshal_core_inputs(x, h_prev, c_prev, delta_t, w64_np, w16_np, lo, hi):
    """Build one core's input map from a batch slice [lo, hi)."""
    rows = hi - lo
    nm = rows // 128
    xs = np.ascontiguousarray(x[lo:hi].T.astype(np.float16))
    hs = np.empty((D_H + 1, rows), np.float16)
    hs[:D_H] = h_prev[lo:hi].T
    hs[D_H] = 1.0  # bias row
    # device row (p, jcol) <-> original row jcol*128 + p
    cps = np.ascontiguousarray(
        c_prev[lo:hi].astype(np.float16).reshape(nm, 128, D_H).transpose(1, 0, 2)
    )
    dts = np.ascontiguousarray(delta_t[lo:hi].astype(np.float16).reshape(nm, 128).T)
    return {"xT": xs, "hT": hs, "cp": cps, "dt": dts, "w64": w64_np, "w16": w16_np}


def unmarshal_output(dev_out, rows):
    """[128, nm, 32] packed fp16 -> ([rows,16], [rows,16]) fp32 batch-major."""
    out = np.asarray(dev_out, np.float32).transpose(1, 0, 2).reshape(rows, 2 * D_H)
    return np.ascontiguousarray(out[:, :D_H]), np.ascontiguousarray(out[:, D_H:])


_PROGRAM_CACHE = {}


def _get_program(rows, mega, chunk):
    key = (rows, mega, chunk)
    if key not in _PROGRAM_CACHE:
        _PROGRAM_CACHE[key] = build_program(rows, mega, chunk)
    return _PROGRAM_CACHE[key]


def run(x, h_prev, c_prev, delta_t, w64_np, w16_np, rows_per_core, mega, chunk,
        trace=False):
    nc = _get_program(rows_per_core, mega, chunk)
    n_cores = N_CORES
    in_maps = [
        marshal_core_inputs(
            x, h_prev, c_prev, delta_t, w64_np, w16_np,
            i * rows_per_core, (i + 1) * rows_per_core,
        )
        for i in range(n_cores)
    ]
    res = run_bass_kernel_spmd(nc, in_maps, list(range(n_cores)), trace=trace)
    parts = [unmarshal_output(res.results[i]["hc"], rows_per_core) for i in range(n_cores)]
    h_next = np.concatenate([p[0] for p in parts], axis=0)
    c_next = np.concatenate([p[1] for p in parts], axis=0)
    return (h_next, c_next), res


def make_weights(W_i, b_i, W_f, b_f, W_o, b_o, W_c, b_c, W_d, b_d):
    """[81,64] fp16 gates block + [81,16] fp16 decay block (bias rows last)."""
    W4 = np.concatenate(
        [np.asarray(w, np.float32) for w in (W_i, W_f, W_o, W_c)], axis=1
    )  # [80, 64]
    b4 = np.concatenate([np.asarray(v, np.float32) for v in (b_i, b_f, b_o, b_c)])
    w64_np = np.ascontiguousarray(
        np.vstack([W4, b4[None, :]]).astype(np.float16)
    )  # [81, 64]
    w16_np = np.ascontiguousarray(
        np.vstack([np.asarray(W_d, np.float32),
                   np.asarray(b_d, np.float32)[None, :]]).astype(np.float16)
    )  # [81, 16]
    return w64_np, w16_np


def kernel(x, h_prev, c_prev, delta_t, W_i, b_i, W_f, b_f, W_o, b_o, W_c, b_c, W_d, b_d):
    x = np.asarray(x, np.float32)
    h_prev = np.asarray(h_prev, np.float32)
    c_prev = np.asarray(c_prev, np.float32)
    delta_t = np.asarray(delta_t, np.float32)
    w64_np, w16_np = make_weights(
        W_i, b_i, W_f, b_f, W_o, b_o, W_c, b_c, W_d, b_d
    )
    (h_next, c_next), _ = run(
        x, h_prev, c_prev, delta_t, w64_np, w16_np,
        rows_per_core=R, mega=32768, chunk=2048,
    )
    return (h_next, c_next)
